# revision 26
# baseline (speedup 1.0000x reference)
"""MoE (top-2 of 8 experts, D=H=1024) on 8 Trainium2 NeuronCores.

Strategy (expert-parallel, matching the sharding hint):
  - Host computes the router (softmax + top-k + expert-sort dispatch) in
    float64 -- the dispatch/sharding decision, 0.2% of total FLOPs.
  - Tokens are gathered per expert (capacity-padded); core c gets expert c's
    token block plus expert c's weights.
  - Each core runs the 2-layer expert MLP in "transposed activation" layout
    (activations are [feature, token]) so no on-device transposes are needed:
        hT = w_in.T @ xT   (lhsT = w_in chunk, natural layout)
        yT = w_out.T @ hT  (lhsT = w_out chunk, natural layout)
    Loops are k-outer so matmuls start as soon as the first weight/activation
    chunks land, with 8 concurrent PSUM accumulation groups per layer.
  - Host scales rows by the gate probability (zero for padding rows) and
    scatter-adds back into the [T, D] output.
"""

import os
import sys

import numpy as np

for _p in ("/opt/trn_rl_repo", "/root/.axon_site/_ro/trn_rl_repo"):
    if os.path.isdir(_p) and _p not in sys.path:
        sys.path.append(_p)


def _ensure_ntff_hook():
    """Register the axon NTFF profiling hook if the image's antenv lacks it."""
    try:
        import antenv.axon_hooks  # noqa: F401

        return
    except ImportError:
        pass
    import types

    try:
        import antenv
    except ImportError:
        return
    mod = types.ModuleType("antenv.axon_hooks")
    _hook = [None]
    mod.set_axon_ntff_profile_hook = lambda h: _hook.__setitem__(0, h)
    mod.get_axon_ntff_profile_hook = lambda: _hook[0]
    sys.modules["antenv.axon_hooks"] = mod
    antenv.axon_hooks = mod
    try:
        from trn_agent_boot.trn_boot import _ntff_profile_via_ctypes

        mod.set_axon_ntff_profile_hook(
            _ntff_profile_via_ctypes("/opt/axon/libaxon_pjrt.so")
        )
    except Exception:
        pass


_ensure_ntff_hook()

D, H, E, TOPK = 1024, 1024, 8, 2
N_CORES = 8
P = 128  # partitions

# Matmul input dtype: float32 (exact, 4 cyc/row), float32r (1 cyc/row,
# ~13-bit multiplies), float16 / bfloat16 (1 cyc/row + fast weight load,
# half the weight DMA bytes).
MM_DTYPE = os.environ.get("MOE_MM_DTYPE", "float16")

_compiled_cache = {}


def _np_mm_dtype(mm_dtype_str):
    if mm_dtype_str in ("float32", "float32r"):
        return np.float32
    if mm_dtype_str == "float16":
        return np.float16
    if mm_dtype_str == "bfloat16":
        import ml_dtypes

        return np.dtype(ml_dtypes.bfloat16)
    raise ValueError(mm_dtype_str)


def _build_program(C, mm_dtype_str):
    """One expert's MLP over a [C] token block; same program on all cores."""
    from concourse import bacc, mybir, tile

    f32 = mybir.dt.float32
    mm_dt = getattr(mybir.dt, mm_dtype_str)
    nc = bacc.Bacc(None, target_bir_lowering=False, debug=False)

    xT_d = nc.dram_tensor("xT", [D, C], mm_dt, kind="ExternalInput")
    w_in_d = nc.dram_tensor("w_in", [D, H], mm_dt, kind="ExternalInput")
    w_out_d = nc.dram_tensor("w_out", [H, D], mm_dt, kind="ExternalInput")
    bias_d = nc.dram_tensor("bias", [2, H], f32, kind="ExternalInput")
    yT_d = nc.dram_tensor("yT", [D, C], f32, kind="ExternalOutput")

    KD = D // P  # contraction chunks, layer 1 (and output chunks, layer 2)
    KH = H // P

    with tile.TileContext(nc) as tc:
        with (
            tc.tile_pool(name="wpool", bufs=1) as wpool,
            tc.tile_pool(name="xpool", bufs=1) as xpool,
            tc.tile_pool(name="hpool", bufs=1) as hpool,
            tc.tile_pool(name="ypool", bufs=1) as ypool,
            tc.tile_pool(name="bpool", bufs=1) as bpool,
            tc.tile_pool(name="psum", bufs=8, space="PSUM") as pspool,
        ):
            w1 = wpool.tile([P, KD, H], mm_dt, tag="w1")
            xt = xpool.tile([P, KD, C], mm_dt, tag="xt")
            w2 = wpool.tile([P, KH, D], mm_dt, tag="w2")
            bias = bpool.tile([P, 2, KH], f32, tag="bias")

            xT_r = xT_d.rearrange("(k p) c -> p k c", p=P)
            # Single SP HWDGE ring, FIFO completion, issued in consumption
            # order.  dma_start costs ~650ns of sequencer time regardless of
            # size, so use small chunks up front (compute starts early) and
            # big chunks later (few issues; transfers pace at HBM rate).
            nc.scalar.dma_start(bias[:], bias_d.rearrange("b (m p) -> p b m", p=P))

            def w_chunk(dst, src, k0, k1):
                src_r = src.rearrange("(k p) h -> p k h", p=P)
                nc.sync.dma_start(dst[:, k0:k1, :], src_r[:, k0:k1, :])

            w1r = w_in_d.rearrange("(k p) h -> p k h", p=P)
            nc.sync.dma_start(w1[:, 0:1, 0:256], w1r[:, 0:1, 0:256])
            nc.sync.dma_start(xt[:, 0:1, :], xT_r[:, 0:1, :])
            nc.sync.dma_start(w1[:, 0:1, 256:], w1r[:, 0:1, 256:])
            nc.sync.dma_start(xt[:, 1:2, :], xT_r[:, 1:2, :])
            w_chunk(w1, w_in_d, 1, 2)
            w_chunk(w1, w_in_d, 2, 4)
            nc.sync.dma_start(xt[:, 2:KD, :], xT_r[:, 2:KD, :])
            w_chunk(w1, w_in_d, 4, 8)
            w_chunk(w2, w_out_d, 0, 4)
            w_chunk(w2, w_out_d, 4, 8)

            gelu = mybir.ActivationFunctionType.Gelu_apprx_tanh

            # PE warmup during the initial DMA window: ~3us of dummy matmuls
            # flips the HAM clock gate to 8/8 before the real stream begins.
            wz = bpool.tile([P, P], mm_dt, tag="wz")
            nc.vector.memset(wz[:], 0.0)
            psw = pspool.tile([P, C], f32, tag="ps", name="ps_warm")
            NWARM = 34
            for i in range(NWARM):
                nc.tensor.matmul(
                    psw[:, :P], wz[:], wz[:], start=(i == 0), stop=(i == NWARM - 1)
                )

            # layer 1, k-outer: 8 concurrent accumulation groups (one/bank)
            ht = hpool.tile([P, KH, C], mm_dt, tag="ht")
            ps1 = [pspool.tile([P, C], f32, tag="ps", name=f"ps1_{i}") for i in range(KH)]
            for k in range(KD):
                for m in range(KH):
                    nc.tensor.matmul(
                        ps1[m][:],
                        w1[:, k, m * P : (m + 1) * P],
                        xt[:, k, :],
                        start=(k == 0),
                        stop=(k == KD - 1),
                    )
            for m in range(KH):
                nc.scalar.activation(
                    ht[:, m, :], ps1[m][:], gelu, bias=bias[:, 0, m : m + 1]
                )

            # layer 2, k-outer
            yt = ypool.tile([P, KD, C], f32, tag="yt")
            ps2 = [pspool.tile([P, C], f32, tag="ps", name=f"ps2_{i}") for i in range(KD)]
            for k in range(KH):
                for m in range(KD):
                    nc.tensor.matmul(
                        ps2[m][:],
                        w2[:, k, m * P : (m + 1) * P],
                        ht[:, k, :],
                        start=(k == 0),
                        stop=(k == KH - 1),
                    )
            # PSUM -> SBUF via plain DVE copies (fastest PSUM drain); the
            # host adds b_out and the gate-probability scaling during the
            # scatter-combine
            yT_r = yT_d.rearrange("(m p) c -> p m c", p=P)
            for m in range(KD):
                nc.vector.tensor_copy(yt[:, m, :], ps2[m][:])
                (nc.scalar if m % 2 else nc.sync).dma_start(
                    yT_r[:, m, :], yt[:, m, :]
                )

    nc.compile()
    if not nc.is_finalized():
        nc.finalize()
    return nc


def _get_program(C):
    key = (C, MM_DTYPE)
    if key not in _compiled_cache:
        _compiled_cache[key] = _build_program(C, MM_DTYPE)
    return _compiled_cache[key]


def _route(x2, router_w):
    """Host router in float64: top-2 experts + gate probs per token."""
    logits = x2.astype(np.float64) @ np.asarray(router_w, np.float64)
    logits -= logits.max(axis=-1, keepdims=True)
    ex = np.exp(logits)
    probs = ex / ex.sum(axis=-1, keepdims=True)
    top_e = np.argsort(-probs, axis=-1, kind="stable")[:, :TOPK]  # [T, K]
    top_p = np.take_along_axis(probs, top_e, axis=-1)  # [T, K]
    return top_e, top_p.astype(np.float32)


def kernel(input_batch, router_w, w_in, b_in, w_out, b_out, run_kwargs=None):
    from concourse.bass_utils import run_bass_kernel_spmd

    x = np.ascontiguousarray(np.asarray(input_batch, np.float32))
    B, S, Dm = x.shape
    T = B * S
    x2 = x.reshape(T, Dm)

    top_e, top_p = _route(x2, router_w)

    # per-expert dispatch lists, in expert-sorted (token, k) order like the
    # reference's stable argsort over flattened (token, k) pairs
    tok_lists = [[] for _ in range(E)]
    p_lists = [[] for _ in range(E)]
    for t in range(T):
        for j in range(TOPK):
            e = top_e[t, j]
            tok_lists[e].append(t)
            p_lists[e].append(top_p[t, j])

    counts = [len(l) for l in tok_lists]
    # capacity per wave; a PSUM bank caps the matmul free dim at 512, so an
    # expert with >512 routed tokens (never happens for the spec'd input
    # distribution) is processed in multiple SPMD waves
    n_waves = max(1, -(-max(counts) // 512))
    if n_waves == 1:
        C = max(256, -(-max(counts) // 4) * 4)  # multiple of 4
    else:
        C = 512

    nc = _get_program(C)
    mm_np = _np_mm_dtype(MM_DTYPE)

    w_in = np.asarray(w_in, np.float32)
    w_out = np.asarray(w_out, np.float32)
    b_in = np.asarray(b_in, np.float32)
    b_out = np.asarray(b_out, np.float32)

    out = np.zeros((T, Dm), np.float32)
    for w in range(n_waves):
        in_maps = []
        for e in range(E):
            idx = np.asarray(tok_lists[e][w * C : (w + 1) * C], np.int64)
            xT = np.zeros((D, C), mm_np)
            if len(idx):
                xT[:, : len(idx)] = x2[idx].T.astype(mm_np)
            in_maps.append(
                {
                    "xT": xT,
                    "w_in": np.ascontiguousarray(w_in[e]).astype(mm_np),
                    "w_out": np.ascontiguousarray(w_out[e]).astype(mm_np),
                    "bias": np.stack([b_in[e], b_out[e]]),
                }
            )

        res = run_bass_kernel_spmd(
            nc, in_maps, core_ids=list(range(N_CORES)), **(run_kwargs or {})
        )
        kernel.last_results = res

        for e in range(E):
            idx = np.asarray(tok_lists[e][w * C : (w + 1) * C], np.int64)
            n = len(idx)
            if n == 0:
                continue
            p = np.asarray(p_lists[e][w * C : (w + 1) * C], np.float32)
            y = (res.results[e]["yT"][:, :n].T + b_out[e]) * p[:, None]
            np.add.at(out, idx, y)

    return out.reshape(B, S, Dm)
